# revision 18
# baseline (speedup 1.0000x reference)
"""TRN2 Bass kernel for nn_Basicblock (binarized CNN basic block).

Strategy: data-parallel over batch (4 images per core x 8 cores).
Binary convs (+-1 activations x +-1 weights) run as exact fp8/bf16 matmuls
with fp32 PSUM accumulation. Training-mode BN uses global batch stats via
two tiny on-device AllReduces. BN affine, biases and PReLU are folded into
per-channel scalar_tensor_tensor + ACT Prelu ops.
"""
import os
import sys

sys.path.insert(0, "/opt/trn_rl_repo")
os.environ.setdefault("MYCRO_LOCAL_CACHE", "1")

import numpy as np

import concourse.bass as bass
import concourse.mybir as mybir
import concourse.tile as tile
from concourse import bacc, bass_utils
from contextlib import ExitStack

F32 = mybir.dt.float32
F16 = mybir.dt.float16
F8 = mybir.dt.float8e4
AF = mybir.ActivationFunctionType
ALU = mybir.AluOpType

NCORES = 8
P = 128
IMGS = 4          # images per core
H = W = 56
HP = 58           # padded spatial
PIXI = H * W      # 3136
PIXC = IMGS * PIXI  # 12544 pixels per channel-chunk per core
RG = 8            # output rows per matmul group
NG = H // RG      # 7 groups per image
NMM = RG * W      # 448 matmul free dim
NT = IMGS * NG    # 28 psum tiles per oc-chunk (conv1)
NB2 = PIXC // NMM  # 28 pixel blocks (conv2)
QB = 784          # elementwise block (quarter image)
NQ = PIXC // QB   # 16 elementwise blocks per chunk
NTOT = float(32 * PIXI)
EPS = 1e-5
NCON = 11

_nc_cache = {}


def _build():
    nc = bacc.Bacc("TRN2", target_bir_lowering=False, debug=False,
                   enable_asserts=False, num_devices=NCORES)
    DR = mybir.MatmulPerfMode.DoubleRow
    x_d = nc.dram_tensor("x", [IMGS, 256, H, W], F32, kind="ExternalInput").ap()
    w1_d = nc.dram_tensor("w1", [P, 18, 2, P], F8, kind="ExternalInput").ap()
    w2_d = nc.dram_tensor("w2", [P, 2, 2, P], F8, kind="ExternalInput").ap()
    cst_d = nc.dram_tensor("consts", [P, 2, NCON], F32, kind="ExternalInput").ap()
    out_d = nc.dram_tensor("out", [IMGS, 256, H, W], F32, kind="ExternalOutput").ap()

    def x_flat(img, cc):
        return x_d[img, cc * P:(cc + 1) * P, :, :].rearrange("c h w -> c (h w)")

    def out_flat(img, oc):
        return out_d[img, oc * P:(oc + 1) * P, :, :].rearrange("c h w -> c (h w)")

    with tile.TileContext(nc) as tc, ExitStack() as ctx:
        kp = ctx.enter_context(tc.tile_pool(name="kp", bufs=1))
        smp = ctx.enter_context(tc.tile_pool(name="smp", bufs=24))
        pp = ctx.enter_context(tc.tile_pool(name="pp", bufs=2))
        yy = ctx.enter_context(tc.tile_pool(name="yy", bufs=2))
        big8 = ctx.enter_context(tc.tile_pool(name="big8", bufs=1))
        xbp = ctx.enter_context(tc.tile_pool(name="xbp", bufs=3))
        vp = ctx.enter_context(tc.tile_pool(name="vp", bufs=5))
        psp = ctx.enter_context(tc.tile_pool(name="psp", bufs=6, space="PSUM"))
        drp = ctx.enter_context(tc.tile_pool(name="drp", bufs=1, space="DRAM"))

        cst = kp.tile([P, 2, NCON], F32, name="cst")
        nc.sync.dma_start(cst[:], cst_d)
        w1s = kp.tile([P, 18, 2, P], F8, name="w1s")
        nc.sync.dma_start(w1s[:], w1_d)
        w2s = kp.tile([P, 2, 2, P], F8, name="w2s")
        nc.sync.dma_start(w2s[:], w2_d)

        p_t = [pp.tile([P, PIXC], F32, name=f"p{c}", tag="p") for c in (0, 1)]
        y1_t = [yy.tile([P, PIXC], F16, name=f"y1_{o}", tag="y") for o in (0, 1)]
        xpad = big8.tile([P, 2, IMGS, HP, HP], F8, name="xpad", tag="b8")
        bnst1 = [kp.tile([P, NT, 6], F32, name=f"bnst1_{o}") for o in (0, 1)]
        bnst2 = [kp.tile([P, NB2, 6], F32, name=f"bnst2_{o}") for o in (0, 1)]
        a1p = [kp.tile([P, 1], F32, name=f"a1p{o}") for o in (0, 1)]
        c1b = [kp.tile([P, 1], F32, name=f"c1b{o}") for o in (0, 1)]
        a2p = [kp.tile([P, 1], F32, name=f"a2p{o}") for o in (0, 1)]
        c2b = [kp.tile([P, 1], F32, name=f"c2b{o}") for o in (0, 1)]

        # ---------------- phase A: pad borders, load x, sign -> xpad (fp8)
        nc.gpsimd.memset(xpad[:, :, :, 0, :], 0.0)
        nc.gpsimd.memset(xpad[:, :, :, HP - 1, :], 0.0)
        nc.gpsimd.memset(xpad[:, :, :, :, 0], 0.0)
        nc.gpsimd.memset(xpad[:, :, :, :, HP - 1], 0.0)
        for img in range(IMGS):
            for c in (0, 1):
                for q in range(4):
                    xs = xbp.tile([P, 14, W], F32, tag="xr", name="xs")
                    nc.sync.dma_start(
                        xs[:], x_flat(img, c)[:, q * QB:(q + 1) * QB]
                        .rearrange("c (h w) -> c h w", w=W))
                    nc.scalar.activation(
                        xpad[:, c, img, 1 + 14 * q:15 + 14 * q, 1:57], xs[:],
                        AF.Sign, bias=cst[:, c, 0:1])

        # ---------------- conv1 helpers (DoubleRow: K=256 per matmul)
        def conv1_group(oc, img, g):
            def emit():
                ps = psp.tile([P, NMM], F32, tag="ps1", name="ps1", bufs=4)
                for kidx in range(9):
                    dh, dw = divmod(kidx, 3)
                    nc.tensor.matmul(
                        ps[:],
                        w1s[:, oc * 9 + kidx, :, :],
                        xpad[:, :, img, g * RG + dh:g * RG + RG + dh,
                             dw:dw + W],
                        start=(kidx == 0), stop=(kidx == 8),
                        perf_mode=DR)
                ti = img * NG + g
                sl = slice(img * PIXI + g * NMM, img * PIXI + (g + 1) * NMM)
                nc.scalar.activation(y1_t[oc][:, sl], ps[:], AF.Copy)
                nc.vector.bn_stats(bnst1[oc][:, ti, :], ps[:])
            return emit

        def emit_stats(bnst, tag):
            agg = kp.tile([P, 2], F32, name=f"agg_{tag}")
            nc.vector.bn_aggr(agg[:], bnst[:].rearrange("p a b -> p (a b)"))
            pk = kp.tile([P, 2], F32, name=f"pk_{tag}")
            nc.vector.tensor_scalar_mul(pk[:, 0:1], agg[:, 0:1], float(PIXC))
            msq = smp.tile([P, 1], F32, tag="sm", name="sm")
            nc.vector.tensor_tensor(msq[:], agg[:, 0:1], agg[:, 0:1], ALU.mult)
            t2 = smp.tile([P, 1], F32, tag="sm", name="sm")
            nc.vector.tensor_tensor(t2[:], agg[:, 1:2], msq[:], ALU.add)
            nc.vector.tensor_scalar_mul(pk[:, 1:2], t2[:], float(PIXC))
            cin = drp.tile([P, 2], F32, name=f"cin_{tag}")
            cout = drp.tile([P, 2], F32, name=f"cout_{tag}", addr_space="Shared")
            nc.sync.dma_start(cin[:], pk[:])
            nc.gpsimd.collective_compute(
                "AllReduce", ALU.add, replica_groups=[list(range(NCORES))],
                ins=[cin.opt()], outs=[cout.opt()])
            gsb = kp.tile([P, 2], F32, name=f"gst_{tag}")
            nc.sync.dma_start(gsb[:], cout[:])
            return gsb

        def stats1(oc):
            return emit_stats(bnst1[oc], f"1{oc}")

        def coef_math(gsb_s1, gsb_s2, oc, a_t, c_t, j_s2, j_gs, j_cb):
            # a = gs / sqrt(s^2*var_raw + eps); c = cb - a*mean_raw
            m = smp.tile([P, 1], F32, tag="sm", name="sm")
            nc.vector.tensor_scalar_mul(m[:], gsb_s1, 1.0 / NTOT)
            e2 = smp.tile([P, 1], F32, tag="sm", name="sm")
            nc.vector.tensor_scalar_mul(e2[:], gsb_s2, 1.0 / NTOT)
            msq = smp.tile([P, 1], F32, tag="sm", name="sm")
            nc.vector.tensor_tensor(msq[:], m[:], m[:], ALU.mult)
            vr = smp.tile([P, 1], F32, tag="sm", name="sm")
            nc.vector.tensor_tensor(vr[:], e2[:], msq[:], ALU.subtract)
            ve = smp.tile([P, 1], F32, tag="sm", name="sm")
            nc.vector.tensor_scalar(
                out=ve[:], in0=vr[:], scalar1=cst[:, oc, j_s2:j_s2 + 1],
                scalar2=EPS, op0=ALU.mult, op1=ALU.add)
            sd = smp.tile([P, 1], F32, tag="sm", name="sm")
            nc.scalar.activation(sd[:], ve[:], AF.Sqrt)
            inv = smp.tile([P, 1], F32, tag="sm", name="sm")
            nc.vector.reciprocal(inv[:], sd[:])
            nc.vector.tensor_scalar_mul(a_t[:], inv[:], cst[:, oc, j_gs:j_gs + 1])
            am = smp.tile([P, 1], F32, tag="sm", name="sm")
            nc.vector.tensor_tensor(am[:], a_t[:], m[:], ALU.mult)
            nc.vector.tensor_tensor(c_t[:], cst[:, oc, j_cb:j_cb + 1], am[:],
                                    ALU.subtract)

        def phaseB_block(oc, i):
            def emit():
                img, q = divmod(i, 4)
                sl = slice(img * PIXI + q * QB, img * PIXI + (q + 1) * QB)
                xr = xbp.tile([P, QB], F32, tag="xr", name="xr")
                nc.sync.dma_start(xr[:], x_flat(img, oc)[:, q * QB:(q + 1) * QB])
                t = vp.tile([P, QB], F32, tag="v", name="t")
                nc.vector.tensor_scalar(
                    out=t[:], in0=y1_t[oc][:, sl], scalar1=a1p[oc][:],
                    scalar2=c1b[oc][:], op0=ALU.mult, op1=ALU.add)
                nc.vector.tensor_tensor(t[:], t[:], xr[:], ALU.add)
                nc.scalar.activation(p_t[oc][:, sl], t[:], AF.Prelu,
                                     bias=0.0, alpha=cst[:, oc, 4:5])
            return emit

        # conv1 oc0
        for img in range(IMGS):
            for g in range(NG):
                conv1_group(0, img, g)()
        gsb0 = stats1(0)

        # conv1 oc1 interleaved with coef math + phase B oc0
        extras = [lambda: coef_math(gsb0[:, 0:1], gsb0[:, 1:2], 0,
                                    a1p[0], c1b[0], 2, 1, 3)]
        extras += [phaseB_block(0, i) for i in range(NQ)]
        groups = [conv1_group(1, img, g) for img in range(IMGS)
                  for g in range(NG)]
        ei = 0
        for i, grp in enumerate(groups):
            grp()
            while ei < len(extras) and ei <= (i * len(extras)) // len(groups):
                extras[ei]()
                ei += 1
        while ei < len(extras):
            extras[ei]()
            ei += 1
        gsb1 = stats1(1)
        coef_math(gsb1[:, 0:1], gsb1[:, 1:2], 1, a1p[1], c1b[1], 2, 1, 3)

        # phase B oc1 interleaved with sign2 (p -> xb2 fp8, reuses xpad slot)
        xb2 = big8.tile([P, 2, PIXC], F8, name="xb2", tag="b8")
        y2_t = [yy.tile([P, PIXC], F16, name=f"y2_{o}", tag="y") for o in (0, 1)]

        def sign2_block(c, img):
            sl = slice(img * PIXI, (img + 1) * PIXI)
            nc.scalar.activation(xb2[:, c, sl], p_t[c][:, sl], AF.Sign,
                                 bias=cst[:, c, 5:6])

        for img in range(IMGS):
            for q in range(4):
                phaseB_block(1, img * 4 + q)()
            sign2_block(1, img)
            sign2_block(0, img)

        # ---------------- conv2 (1x1, DoubleRow K=256)
        def conv2_block(oc, pb):
            def emit():
                sl = slice(pb * NMM, (pb + 1) * NMM)
                ps = psp.tile([P, NMM], F32, tag="ps2", name="ps2", bufs=4)
                nc.tensor.matmul(ps[:], w2s[:, oc, :, :], xb2[:, :, sl],
                                 start=True, stop=True, perf_mode=DR)
                nc.scalar.activation(y2_t[oc][:, sl], ps[:], AF.Copy)
                nc.vector.bn_stats(bnst2[oc][:, pb, :], ps[:])
            return emit

        def phaseD_block(oc, i):
            def emit():
                img, q = divmod(i, 4)
                sl = slice(img * PIXI + q * QB, img * PIXI + (q + 1) * QB)
                t = vp.tile([P, QB], F32, tag="v", name="td")
                nc.vector.tensor_scalar(
                    out=t[:], in0=y2_t[oc][:, sl], scalar1=a2p[oc][:],
                    scalar2=c2b[oc][:], op0=ALU.mult, op1=ALU.add)
                nc.vector.tensor_tensor(t[:], t[:], p_t[oc][:, sl], ALU.add)
                pr = xbp.tile([P, QB], F32, tag="xr", name="pr")
                nc.scalar.activation(pr[:], t[:], AF.Prelu, bias=0.0,
                                     alpha=cst[:, oc, 9:10])
                nc.scalar.activation(t[:], pr[:], AF.Identity,
                                     bias=cst[:, oc, 10:11])
                nc.sync.dma_start(out_flat(img, oc)[:, q * QB:(q + 1) * QB],
                                  t[:])
            return emit

        for pb in range(NB2):
            conv2_block(0, pb)()
        g2a = emit_stats(bnst2[0], "20")

        # conv2 oc1 interleaved with coef math + phase D oc0
        extras2 = [lambda: coef_math(g2a[:, 0:1], g2a[:, 1:2], 0,
                                     a2p[0], c2b[0], 7, 6, 8)]
        extras2 += [phaseD_block(0, i) for i in range(NQ)]
        groups2 = [conv2_block(1, pb) for pb in range(NB2)]
        ei = 0
        for i, grp in enumerate(groups2):
            grp()
            while ei < len(extras2) and ei <= (i * len(extras2)) // len(groups2):
                extras2[ei]()
                ei += 1
        while ei < len(extras2):
            extras2[ei]()
            ei += 1
        g2b = emit_stats(bnst2[1], "21")
        coef_math(g2b[:, 0:1], g2b[:, 1:2], 1, a2p[1], c2b[1], 7, 6, 8)
        for i in range(NQ):
            phaseD_block(1, i)()

    nc.compile()
    return nc


def _get_nc():
    if "nc" not in _nc_cache:
        _nc_cache["nc"] = _build()
    return _nc_cache["nc"]


def _prep_inputs(inputs):
    f8np = mybir.dt.np(F8)
    x = np.ascontiguousarray(np.asarray(inputs["x"], np.float32))
    w3 = np.asarray(inputs["w3x3"], np.float32)
    wr = np.asarray(inputs["wres"], np.float32)
    s1 = np.abs(w3).mean(axis=(1, 2, 3))
    s2 = np.abs(wr).mean(axis=(1, 2, 3))
    w1h = (np.sign(w3).reshape(2, P, 2, P, 3, 3).transpose(3, 0, 4, 5, 2, 1)
           .reshape(P, 18, 2, P)).astype(f8np)
    w2h = (np.sign(wr)[:, :, 0, 0].reshape(2, P, 2, P).transpose(3, 0, 2, 1)
           .reshape(P, 2, 2, P)).astype(f8np)

    def col(v):
        return np.asarray(v, np.float32).reshape(2, P).T

    g1 = np.asarray(inputs["bn1_g"], np.float32)
    be1 = np.asarray(inputs["bn1_b"], np.float32)
    g2 = np.asarray(inputs["bn2_g"], np.float32)
    be2 = np.asarray(inputs["bn2_b"], np.float32)
    b1_1, b1_2, b1_3 = (np.asarray(inputs[k], np.float32)
                        for k in ("b1_1", "b1_2", "b1_3"))
    b2_1, b2_2, b2_3 = (np.asarray(inputs[k], np.float32)
                        for k in ("b2_1", "b2_2", "b2_3"))
    pa1 = np.asarray(inputs["prelu1_a"], np.float32)
    pa2 = np.asarray(inputs["prelu2_a"], np.float32)
    cols = [b1_1, g1 * s1, s1 * s1, be1 + b1_2, pa1, b1_3 + b2_1,
            g2 * s2, s2 * s2, be2 + b1_3 + b2_2, pa2, b2_3]
    csth = np.stack([col(v) for v in cols], axis=2).astype(np.float32)
    csth = np.ascontiguousarray(csth)

    in_maps = []
    for c in range(NCORES):
        in_maps.append({
            "x": np.ascontiguousarray(x[c * IMGS:(c + 1) * IMGS]),
            "w1": w1h, "w2": w2h, "consts": csth,
        })
    return in_maps


def _run(in_maps, trace=False):
    nc = _get_nc()
    return bass_utils.run_bass_kernel_spmd(
        nc, in_maps, core_ids=list(range(NCORES)), trace=trace)


def kernel(**inputs):
    in_maps = _prep_inputs(inputs)
    res = _run(in_maps)
    out = np.concatenate([res.results[c]["out"] for c in range(NCORES)], axis=0)
    return out.astype(np.float32)
